# revision 60
# baseline (speedup 1.0000x reference)
"""DCNv3 forward on 8 trn2 NeuronCores.

Strategy (data-parallel over (batch, H-half) -> 8 shards), v2:

The v1 kernel was DVE-bound: 20K tiny (FD=32) scalar_tensor_tensor ops
applying per-(g,p,i,j) tap coefficients pixel-column by pixel-column.
v2 restructures the whole coefficient pipeline into a
"taps-on-partitions, pixels-on-free" layout so every elementwise op has
FD=512, and uses the PE for all cross-partition data movement:

  - praw matmuls produce per-TAP duplicated offset rows directly
    (weight matrix columns are duplicated per tap on the host)
  - hat weights: 2 ACT ops per chunk (bias folds the per-tap integer)
  - softmax: exp on ACT, group-sum + reciprocal-replicate via tiny
    0/1-indicator matmuls on PE
  - tap coefficients el*haty*hatx: 2 big TT ops per chunk (fp16, 2x)
  - tap -> (cell,g) collapse: one PE matmul with a constant 0/1 matrix
    (cells = distinct (dy,s) integer sample shifts; 321 taps -> 103
    live (g,cell) pairs packed into 128 rows = 32 cells x 4 groups)
  - PE transpose moves Band to [pixel-column, (row, cell)] layout
  - apply: per cell, ONE fp16 TT mult (coef broadcast over channels)
    + ONE fp16 TT add over all RT rows x contiguous-group channels
    (~72 ops of FD~=2-8K per 16-row tile instead of ~5100 FD=32 ops)

Everything runs in fp16 (DVE 2x mode where APs allow); accumulation
inside matmuls is fp32 (PSUM).
"""

import numpy as np
import sys

sys.path.insert(0, "/opt/trn_rl_repo")

import concourse.bass as bass
import concourse.bacc as bacc
import concourse.mybir as mybir
import concourse.tile as tile
from concourse.bass_utils import run_bass_kernel_spmd

B, C, H, W = 4, 128, 128, 128
G, P, gc = 4, 9, 32
N_CORES = 8
HS = H // 2          # rows per core (b, half)
RT = 16              # output rows per device tile
NTILES = HS // RT    # 4
NQ = RT * W // 512   # 512-pixel chunks per tile (PSUM bank = 512 fp32)

f32 = mybir.dt.float32
f16 = mybir.dt.float16

_KS = np.array([-1.0, 0.0, 1.0], np.float32)
KX = np.repeat(_KS, 3)   # x-major flatten (matches torch meshgrid in ref)
KY = np.tile(_KS, 3)


class _Geom:
    pass


def _make_geom(inp, W_off, b_off):
    """Integer tap geometry from the actual offset field (host-side)."""
    g = _Geom()
    xhw = inp.reshape(B, H, W, C)
    off = (xhw.reshape(-1, C) @ W_off + b_off).reshape(-1, G, P, 2)
    rx = off[..., 0] + KX
    ry = off[..., 1] + KY
    g.Bx = np.floor(rx.min(axis=0)).astype(np.int64)
    g.By = np.floor(ry.min(axis=0)).astype(np.int64)
    g.spx = np.minimum(np.floor(rx.max(axis=0)).astype(np.int64) + 2 - g.Bx, 4)
    g.spy = np.minimum(np.floor(ry.max(axis=0)).astype(np.int64) + 2 - g.By, 4)

    g.taps = [(gg, p, i, j) for gg in range(G) for p in range(P)
              for i in range(int(g.spy[gg, p])) for j in range(int(g.spx[gg, p]))]
    g.T = len(g.taps)
    cells = sorted({(int(g.By[gg, p]) + i, int(g.Bx[gg, p]) + j)
                    for (gg, p, i, j) in g.taps})
    assert len(cells) <= 32, f"too many cells: {len(cells)}"
    g.cells = cells
    g.NCELL = len(cells)
    cidx = {c: k for k, c in enumerate(cells)}
    g.kappa = [cidx[(int(g.By[gg, p]) + i, int(g.Bx[gg, p]) + j)] * 4 + gg
               for (gg, p, i, j) in g.taps]
    # group-contiguous apply runs per cell: (cellidx, dy, s, g0, ng)
    livemask = set(g.kappa)
    g.runs = []
    for k, (dy, s) in enumerate(cells):
        gg = 0
        while gg < G:
            if k * 4 + gg in livemask:
                g0 = gg
                while gg < G and k * 4 + gg in livemask:
                    gg += 1
                g.runs.append((k, dy, s, g0, gg - g0))
            else:
                gg += 1

    full = [r for r in g.runs if r[4] == G]
    part = [r for r in g.runs if r[4] < G]
    # engine balance: keep PE_CELLS full cells on the PE accumulate path,
    # push the rest onto the DVE side-accumulator (measured ~1:1 us trade)
    PE_CELLS = 19
    g.full = full[:PE_CELLS]
    g.part = part + full[PE_CELLS:]
    g.dymin = min(c[0] for c in cells)
    g.dymax = max(c[0] for c in cells)
    g.smin = min(c[1] for c in cells)
    g.smax = max(c[1] for c in cells)
    g.NS = g.smax - g.smin + 1
    g.C0 = 3 + g.smin            # slab col = wo + (s - smin) + C0 + ... = wo+s+3
    assert g.C0 >= 0
    g.NR = RT + (g.dymax - g.dymin)  # slab rows per tile window
    g.NROW = RT * (NTILES - 1) + g.NR + 1
    g.NCOL = W + g.NS - 1 + g.C0 + 1
    # tap chunks of <=128 rows
    g.nch = (g.T + 127) // 128
    g.csz = [(g.T + g.nch - 1) // g.nch] * g.nch
    g.csz[-1] = g.T - sum(g.csz[:-1])
    g.cof = [sum(g.csz[:c]) for c in range(g.nch)]
    return g


def _build(g: "_Geom"):
    nc = bacc.Bacc("TRN2", target_bir_lowering=False, debug=False,
                   num_devices=N_CORES)

    xslab_t = nc.dram_tensor("xslab", [g.NROW * g.NCOL * C], f16, kind="ExternalInput")
    xchw_t = nc.dram_tensor("xchw", [C, HS * W], f16, kind="ExternalInput")
    # packed constant blobs (fp16 matrices + fp32 bias columns)
    wy_t = nc.dram_tensor("wy", [C, g.nch * 128], f16, kind="ExternalInput")
    wx_t = nc.dram_tensor("wx", [C, g.nch * 128], f16, kind="ExternalInput")
    wlog_t = nc.dram_tensor("wlog", [C, G * P], f16, kind="ExternalInput")
    indrep_t = nc.dram_tensor("indrep", [G * P, g.nch * 128], f16, kind="ExternalInput")
    indden_t = nc.dram_tensor("indden", [G * P, G], f16, kind="ExternalInput")
    indr36_t = nc.dram_tensor("indr36", [G, G * P], f16, kind="ExternalInput")
    coll_t = nc.dram_tensor("coll", [128, g.nch * 128], f16, kind="ExternalInput")
    ident_t = nc.dram_tensor("ident", [128, 128], f16, kind="ExternalInput")
    bias_t = nc.dram_tensor("bias", [128, 2 * g.nch + 2], f32, kind="ExternalInput")
    out_t = nc.dram_tensor("out", [HS * W * C], f16, kind="ExternalOutput")

    NS, NR, nch = g.NS, g.NR, g.nch
    mult, add = mybir.AluOpType.mult, mybir.AluOpType.add
    AF = mybir.ActivationFunctionType

    def vap(v, off, dims):
        return bass.AP(tensor=v.tensor, offset=v.offset + off, ap=[v.ap[0]] + dims)

    with tile.TileContext(nc) as tc:
        with (
            tc.tile_pool(name="const", bufs=1) as cpool,
            tc.tile_pool(name="xs", bufs=2) as xspool,
            tc.tile_pool(name="work", bufs=2) as wpool,
            tc.tile_pool(name="tmpp", bufs=6) as tpool,
            tc.tile_pool(name="hot", bufs=3) as hpool,
            tc.tile_pool(name="psmm", bufs=3, space="PSUM") as pmm,
            tc.tile_pool(name="psbt", bufs=1, space="PSUM") as pbt,
            tc.tile_pool(name="psacc", bufs=1, space="PSUM") as pacc,
        ):
            # ---- constants: DMA land, then ACT copy (matmul single-sem rule)
            def cload(name, shape, dt, src_ap):
                t0 = cpool.tile(shape, dt, name=name + "0")
                nc.sync.dma_start(t0[:], src_ap)
                t1 = cpool.tile(shape, dt, name=name)
                nc.scalar.copy(t1[:], t0[:])
                return t1

            wy = cload("wy", [C, nch * 128], f16, wy_t.ap())
            wx = cload("wx", [C, nch * 128], f16, wx_t.ap())
            wlog = cload("wlog", [C, G * P], f16, wlog_t.ap())
            indrep = cload("indrep", [G * P, nch * 128], f16, indrep_t.ap())
            indden = cload("indden", [G * P, G], f16, indden_t.ap())
            indr36 = cload("indr36", [G, G * P], f16, indr36_t.ap())
            coll = cload("coll", [128, nch * 128], f16, coll_t.ap())
            ident = cload("ident", [128, 128], f16, ident_t.ap())
            biasc = cpool.tile([128, 2 * nch + 2], f32, name="biasc")
            nc.sync.dma_start(biasc[:], bias_t.ap())

            def emit_front(t):
                # ---- loads ----------------------------------------------
                xs = xspool.tile([C, NS * NR * C], f16, name="xs")
                for si in range(NS):
                    src = bass.AP(
                        tensor=xslab_t,
                        offset=(RT * t * g.NCOL + g.C0 + si) * C,
                        ap=[[C, W], [g.NCOL * C, NR], [1, C]])
                    nc.sync.dma_start(
                        vap(xs[:], si * NR * C, [[C, NR], [1, C]]), src)

                xc0 = wpool.tile([C, RT * W], f16, name="xc0")
                nc.sync.dma_start(
                    xc0[:], bass.AP(tensor=xchw_t, offset=RT * t * W,
                                    ap=[[HS * W, C], [1, RT * W]]))
                xc = wpool.tile([C, RT * W], f16, name="xc")
                nc.scalar.copy(xc[:], xc0[:])

                bandT = wpool.tile([C, RT * 256], f16, name="bandT")

                for q in range(NQ):
                    xcq = xc[:, q * 512:(q + 1) * 512]

                    # softmax over P, normalized el rows ------------------
                    lg_ps = pmm.tile([128, 512], f32, name="mm")
                    nc.tensor.matmul(lg_ps[:G * P], wlog[:], xcq,
                                     start=True, stop=True)
                    el36 = wpool.tile([G * P, 512], f16, name="el36")
                    nc.scalar.activation(el36[:], lg_ps[:G * P], AF.Exp,
                                         bias=biasc[:G * P, 2 * nch:2 * nch + 1])
                    den_ps = pmm.tile([128, 512], f32, name="mm")
                    nc.tensor.matmul(den_ps[:G], indden[:], el36[:],
                                     start=True, stop=True)
                    denr = wpool.tile([G, 512], f32, name="denr")
                    nc.vector.reciprocal_approx_fast(denr[:], den_ps[:G])
                    denrh = wpool.tile([G, 512], f16, name="denrh")
                    nc.scalar.copy(denrh[:], denr[:])
                    d36_ps = pmm.tile([128, 512], f32, name="mm")
                    nc.tensor.matmul(d36_ps[:G * P], indr36[:], denrh[:],
                                     start=True, stop=True)
                    d36 = wpool.tile([G * P, 512], f16, name="d36s")
                    nc.scalar.copy(d36[:], d36_ps[:G * P])
                    el36n = wpool.tile([G * P, 512], f16, name="el36n")
                    nc.vector.tensor_tensor(el36n[:], el36[:], d36[:], mult)

                    # per-chunk: offsets -> hats -> tap coefs -> collapse -
                    band_ps = pbt.tile([128, 512], f32, name="bt")
                    for ch in range(nch):
                        m = g.csz[ch]
                        oy_ps = pmm.tile([128, 512], f32, name="mm")
                        nc.tensor.matmul(oy_ps[:m], wy[:, ch * 128:ch * 128 + m],
                                         xcq, start=True, stop=True)
                        hy = hpool.tile([128, 512], f16, name="hy")
                        nc.scalar.activation(hy[:m], oy_ps[:m], AF.Abs,
                                             bias=biasc[:m, ch:ch + 1])
                        nc.scalar.activation(hy[:m], hy[:m], AF.Relu,
                                             bias=biasc[:m, 2 * nch + 1:2 * nch + 2],
                                             scale=-1.0)
                        ox_ps = pmm.tile([128, 512], f32, name="mm")
                        nc.tensor.matmul(ox_ps[:m], wx[:, ch * 128:ch * 128 + m],
                                         xcq, start=True, stop=True)
                        hx = hpool.tile([128, 512], f16, name="hx")
                        nc.scalar.activation(hx[:m], ox_ps[:m], AF.Abs,
                                             bias=biasc[:m, nch + ch:nch + ch + 1])
                        nc.scalar.activation(hx[:m], hx[:m], AF.Relu,
                                             bias=biasc[:m, 2 * nch + 1:2 * nch + 2],
                                             scale=-1.0)
                        er_ps = pmm.tile([128, 512], f32, name="mm")
                        nc.tensor.matmul(er_ps[:m],
                                         indrep[:, ch * 128:ch * 128 + m],
                                         el36n[:], start=True, stop=True)
                        er = hpool.tile([128, 512], f16, name="er")
                        nc.scalar.copy(er[:m], er_ps[:m])
                        tp = hpool.tile([128, 512], f16, name="tp")
                        nc.vector.tensor_tensor(tp[:m], hy[:m], hx[:m], mult)
                        nc.vector.tensor_tensor(tp[:m], tp[:m], er[:m], mult)
                        nc.tensor.matmul(band_ps[:],
                                         coll[:m, ch * 128:(ch + 1) * 128],
                                         tp[:m], start=(ch == 0),
                                         stop=(ch == nch - 1))

                    band_sb = wpool.tile([128, 512], f16, name="bandsb")
                    nc.scalar.copy(band_sb[:], band_ps[:])
                    # transpose to [wo, cell] per output row (4 slices of
                    # one PSUM tile), then ONE copy that duplicates each
                    # coef x2 so the apply-mult innermost AP dim is
                    # stride-1 pairs (enables the DVE 2x perf mode)
                    tr_ps = pbt.tile([128, 512], f16, name="bt")
                    for r in range(4):
                        nc.tensor.transpose(vap(tr_ps[:], r * 128, [[1, 128]]),
                                            band_sb[:, r * 128:(r + 1) * 128],
                                            ident[:])
                    nc.scalar.copy(
                        vap(bandT[:], q * 4 * 256, [[256, 4], [2, 128], [1, 2]]),
                        vap(tr_ps[:], 0, [[128, 4], [1, 128], [0, 2]]))
                return xs, bandT

            def emit_apply(t, xs, bandT):
                # ---- apply ----------------------------------------------
                # products on DVE ((r,c)-compact tmp, coef read as dup-2
                # pairs); full 4-group cells accumulate via contiguous
                # N=512 identity-matmuls into per-row-chunk PSUM banks;
                # partial-group runs accumulate on DVE into a fp16 side
                # buffer merged by 4 final matmuls.
                acc_ps = [pacc.tile([C, 512], f32, name=f"acc{rc}")
                          for rc in range(4)]

                sacc = wpool.tile([C, RT * C], f16, name="sacc")
                nc.vector.memset(sacc[:], 0)
                for ri, (k, dy, s, g0, ng) in enumerate(g.full):
                    xoff = (((s - g.smin) * NR + (dy - g.dymin)) * C)
                    tmp = tpool.tile([C, RT * C], f16, name="tmp")
                    for gg in range(G):
                        xv = vap(xs[:], xoff + gg * gc, [[C, RT], [1, gc]])
                        cf = vap(bandT[:], (k * 4 + gg) * 2,
                                 [[256, RT], [0, gc // 2], [1, 2]])
                        tv = vap(tmp[:], gg * gc, [[C, RT], [1, gc]])
                        nc.vector.tensor_tensor(tv, xv, cf, mult)
                    for rc in range(4):
                        nc.tensor.matmul(
                            acc_ps[rc][:], ident[:],
                            vap(tmp[:], rc * 512, [[1, 512]]),
                            start=(ri == 0), stop=False)

                for (k, dy, s, g0, ng) in g.part:
                    xoff = (((s - g.smin) * NR + (dy - g.dymin)) * C)
                    tmp = tpool.tile([C, RT * C], f16, name="tmp")
                    for gi in range(ng):
                        gg = g0 + gi
                        xv = vap(xs[:], xoff + gg * gc, [[C, RT], [1, gc]])
                        cf = vap(bandT[:], (k * 4 + gg) * 2,
                                 [[256, RT], [0, gc // 2], [1, 2]])
                        tv = vap(tmp[:], gg * gc, [[C, RT], [1, gc]])
                        nc.vector.tensor_tensor(tv, xv, cf, mult)
                        av = vap(sacc[:], gg * gc, [[C, RT], [1, gc]])
                        nc.vector.tensor_tensor(av, tv, av, add)

                # merge the DVE side accumulator (adds zeros where no
                # partial run touched) and close the accumulation groups
                for rc in range(4):
                    nc.tensor.matmul(acc_ps[rc][:], ident[:],
                                     vap(sacc[:], rc * 512, [[1, 512]]),
                                     start=False, stop=True)

                acc = wpool.tile([C, RT * C], f16, name="acc")
                for rc in range(4):
                    nc.scalar.copy(vap(acc[:], rc * 512, [[1, 512]]),
                                   acc_ps[rc][:])

                nc.sync.dma_start(
                    bass.AP(tensor=out_t, offset=RT * t * W * C,
                            ap=[[C, W], [W * C, RT], [1, C]]),
                    vap(acc[:], 0, [[C, RT], [1, C]]))

            # software pipeline: emit tile t+1's coefficient front before
            # tile t's apply so each engine's in-order stream interleaves
            # [front(t+1)][apply(t)] and the PE never starves on products
            prev = emit_front(0)
            for t in range(1, NTILES):
                cur = emit_front(t)
                emit_apply(t - 1, *prev)
                prev = cur
            emit_apply(NTILES - 1, *prev)

    nc.compile()
    return nc


def _host_prep(inp, W_off, b_off, W_mask, b_mask, g):
    xhw = inp.reshape(B, H, W, C)
    nch, T = g.nch, g.T

    wy = np.zeros((C, nch * 128), np.float16)
    wx = np.zeros((C, nch * 128), np.float16)
    bias = np.zeros((128, 2 * nch + 2), np.float32)
    indrep = np.zeros((G * P, nch * 128), np.float16)
    coll = np.zeros((128, nch * 128), np.float16)
    for t, (gg, p, i, j) in enumerate(g.taps):
        ch, rr = t // g.csz[0], t % g.csz[0]
        gp = gg * P + p
        wy[:, ch * 128 + rr] = W_off[:, 2 * gp + 1]
        wx[:, ch * 128 + rr] = W_off[:, 2 * gp]
        bias[rr, ch] = b_off[2 * gp + 1] + KY[p] - (g.By[gg, p] + i)
        bias[rr, nch + ch] = b_off[2 * gp] + KX[p] - (g.Bx[gg, p] + j)
        indrep[gp, ch * 128 + rr] = 1.0
        coll[rr, ch * 128 + g.kappa[t]] = 1.0
    bias[:G * P, 2 * nch] = b_mask
    bias[:, 2 * nch + 1] = 1.0
    wlog = W_mask.astype(np.float16)
    indden = np.zeros((G * P, G), np.float16)
    for gg in range(G):
        indden[gg * P:(gg + 1) * P, gg] = 1.0
    indr36 = np.zeros((G, G * P), np.float16)
    for gg in range(G):
        indr36[gg, gg * P:(gg + 1) * P] = 1.0
    ident = np.eye(128, dtype=np.float16)

    in_maps = []
    for core in range(N_CORES):
        b, half = divmod(core, 2)
        h0 = HS * half
        xslab = np.zeros((g.NROW, g.NCOL, C), np.float16)
        for lr in range(g.NROW):
            orig = lr + h0 + g.dymin
            if 0 <= orig < H:
                xslab[lr, 3:3 + W, :] = xhw[b, orig]
        xchw = np.ascontiguousarray(
            xhw[b, h0:h0 + HS].reshape(HS * W, C).T).astype(np.float16)
        in_maps.append({
            "xslab": xslab.reshape(-1),
            "xchw": xchw,
            "wy": wy, "wx": wx, "wlog": wlog,
            "indrep": indrep, "indden": indden, "indr36": indr36,
            "coll": coll, "ident": ident, "bias": bias,
        })
    return in_maps


def _run(inp, W_off, b_off, W_mask, b_mask, **spmd_kwargs):
    inp = np.ascontiguousarray(inp, np.float32)
    W_off = np.asarray(W_off, np.float32)
    b_off = np.asarray(b_off, np.float32)
    g = _make_geom(inp, W_off, b_off)
    nc = _build(g)
    in_maps = _host_prep(inp, W_off, b_off,
                         np.asarray(W_mask, np.float32),
                         np.asarray(b_mask, np.float32), g)
    res = run_bass_kernel_spmd(nc, in_maps, core_ids=list(range(N_CORES)),
                               **spmd_kwargs)
    out = np.empty((B, H, W, C), np.float32)
    for core in range(N_CORES):
        b, half = divmod(core, 2)
        out[b, HS * half:HS * (half + 1)] = \
            res.results[core]["out"].astype(np.float32).reshape(HS, W, C)
    return out.reshape(B, C, H, W), res


def kernel(inp, W_off, b_off, W_mask, b_mask):
    out, _ = _run(inp, W_off, b_off, W_mask, b_mask)
    return out


if __name__ == "__main__":
    d = np.load("/root/problem/ref_cache.npz")
    got = kernel(d["inp"], d["W_off"], d["b_off"], d["W_mask"], d["b_mask"])
    exp = d["exp"]
    err = np.abs(got - exp).max()
    print("absmax err:", err, "rel:", err / np.abs(exp).max())
